# revision 1
# baseline (speedup 1.0000x reference)
"""Multi-headed self-attention (B=8, S=1024, D=768, H=12) on 8 TRN2 cores.

Sharding: data-parallel over batch -- core i computes batch element i.
Per-core kernel (all operands pre-transposed on host):
    Qt = (Wq @ x.T + bq)      [D, S]   (o on partitions)
    Kt = (Wk @ x.T + bk)      [D, S]
    V  = (x @ Wv.T + bv)      [S, D]   augmented with a ones column per head
    St_h = Kt_h^T-slices @ Qt_h   -> scores transposed [k, q]
    Et = exp(St/8 + maskbias[k])  (ACT, mask bias per-partition)
    PVt'_h = V'_h.T @ Et_h        [65, q]; row 64 = sum_k Et = Z[q]
    out_h.T = PVt'_h[0:64] / Z    -> outT rows h*64..h*64+63
Host transposes outT back.
"""

import numpy as np

import concourse.bacc as bacc
import concourse.tile as tile
from concourse import mybir
from concourse.bass_utils import run_bass_kernel_spmd

B, S, D, H = 8, 1024, 768, 12
HD = D // H  # 64
N_CORES = 8
SC = S // 128  # 8 key/seq chunks
OC = D // 128  # 6 output chunks (2 heads each)
DC = D // 128  # 6 contraction chunks
NT = 512  # matmul moving-dim tile (fp32 max)
QT = S // NT  # 2
F32 = mybir.dt.float32
F32R = mybir.dt.float32r

HW = HD + 1  # per-head V width incl. ones column


def build():
    nc = bacc.Bacc("TRN2", target_bir_lowering=False, debug=False, num_devices=N_CORES)
    xT = nc.dram_tensor("xT", [D, S], F32R, kind="ExternalInput").ap()
    wqT = nc.dram_tensor("wqT", [D, D], F32R, kind="ExternalInput").ap()
    wkT = nc.dram_tensor("wkT", [D, D], F32R, kind="ExternalInput").ap()
    wvT = nc.dram_tensor("wvT", [D, D], F32R, kind="ExternalInput").ap()
    bq = nc.dram_tensor("bq", [D], F32, kind="ExternalInput").ap()
    bk = nc.dram_tensor("bk", [D], F32, kind="ExternalInput").ap()
    bvb = nc.dram_tensor("bvb", [128, D], F32, kind="ExternalInput").ap()
    mb = nc.dram_tensor("mb", [S], F32, kind="ExternalInput").ap()
    outT = nc.dram_tensor("outT", [D, S], F32, kind="ExternalOutput").ap()

    with tile.TileContext(nc) as tc:
        with (
            tc.tile_pool(name="const", bufs=1) as const,
            tc.tile_pool(name="qk", bufs=2) as qk_pool,
            tc.tile_pool(name="et", bufs=6) as et_pool,
            tc.tile_pool(name="epi", bufs=2) as epi_pool,
            tc.tile_pool(name="st", bufs=3, space="PSUM") as st_ps,
            tc.tile_pool(name="pv", bufs=2, space="PSUM") as pv_ps,
            tc.tile_pool(name="dram", bufs=2, space="DRAM") as dram_pool,
        ):
            # ---------- constant / weight loads ----------
            xt = [const.tile([128, S], F32R, tag=f"xt{c}", name=f"xt{c}") for c in range(DC)]
            wq = [const.tile([128, D], F32R, tag=f"wq{c}", name=f"wq{c}") for c in range(DC)]
            wk = [const.tile([128, D], F32R, tag=f"wk{c}", name=f"wk{c}") for c in range(DC)]
            wv = [const.tile([128, D], F32R, tag=f"wv{c}", name=f"wv{c}") for c in range(DC)]
            # interleave so every d-chunk lands early and evenly
            for c in range(DC):
                nc.sync.dma_start(xt[c][:], xT[c * 128:(c + 1) * 128, :])
                nc.sync.dma_start(wv[c][:], wvT[c * 128:(c + 1) * 128, :])
                nc.sync.dma_start(wq[c][:], wqT[c * 128:(c + 1) * 128, :])
                nc.sync.dma_start(wk[c][:], wkT[c * 128:(c + 1) * 128, :])

            bq_t = const.tile([128, OC], F32, tag="bq")
            nc.sync.dma_start(bq_t[:], bq.rearrange("(c p) -> p c", p=128))
            bk_t = const.tile([128, OC], F32, tag="bk")
            nc.sync.dma_start(bk_t[:], bk.rearrange("(c p) -> p c", p=128))
            bvb_t = const.tile([128, D], F32, tag="bvb")
            nc.sync.dma_start(bvb_t[:], bvb[:])
            mb_t = const.tile([128, SC], F32, tag="mb")
            nc.sync.dma_start(mb_t[:], mb.rearrange("(c p) -> p c", p=128))
            # tiny dummy exp pulls the ~2.7us ACT table load off the
            # critical path (walrus emits the table load before the first
            # ACTIVATE in queue order)
            warm = const.tile([128, 1], F32, tag="warm")
            nc.scalar.activation(
                warm[:], mb_t[:, 0:1], mybir.ActivationFunctionType.Exp
            )

            # ---------- V projection -> vaug [sc][128, H*65] ----------
            vaug = [const.tile([128, H * HW], F32R, tag=f"va{sc}", name=f"va{sc}") for sc in range(SC)]
            for sc in range(SC):
                ones_cols = vaug[sc][:].rearrange("p (h w) -> p h w", h=H)[:, :, HD:HW]
                nc.vector.memset(ones_cols.bitcast(F32), 1.0)
            def v_piece(sc, half):
                n0, n1, h0, h1 = ((0, 512, 0, 8), (512, 768, 8, 12))[half]
                vp = st_ps.tile([128, NT], F32, tag="st", name=f"vp{sc}_{half}")
                for c in range(DC):
                    nc.tensor.matmul(
                        vp[:, : n1 - n0],
                        xt[c][:, sc * 128:(sc + 1) * 128],
                        wv[c][:, n0:n1],
                        start=(c == 0),
                        stop=(c == DC - 1),
                    )
                nc.vector.tensor_add(
                    vaug[sc][:].rearrange("p (h w) -> p h w", h=H)[:, h0:h1, 0:HD],
                    vp[:, : n1 - n0].rearrange("p (h w) -> p h w", w=HD),
                    bvb_t[:, n0:n1].rearrange("p (h w) -> p h w", w=HD),
                )

            # ---------- Q/K projection, emitted in half-projections ----------
            wmap = {"q": (wq, bq_t), "k": (wk, bk_t)}

            def qk_alloc(oc):
                return {
                    name: qk_pool.tile([128, S], F32R, tag=name, name=f"{name}t{oc}")
                    for name in ("q", "k")
                }

            def qk_piece(oc, dsts, name, qt):
                w_t, b_t = wmap[name]
                p = st_ps.tile([128, NT], F32, tag="st", name=f"qkp{name}{qt}")
                for c in range(DC):
                    nc.tensor.matmul(
                        p[:],
                        w_t[c][:, oc * 128:(oc + 1) * 128],
                        xt[c][:, qt * NT:(qt + 1) * NT],
                        start=(c == 0),
                        stop=(c == DC - 1),
                    )
                nc.vector.tensor_scalar_add(
                    dsts[name][:, qt * NT:(qt + 1) * NT], p[:], b_t[:, oc:oc + 1]
                )

            def qk_proj(oc):
                dsts = qk_alloc(oc)
                for name in ("q", "k"):
                    for qt in range(QT):
                        qk_piece(oc, dsts, name, qt)
                return dsts

            # ---------- attention: flat software pipeline, skew=2 ----------
            # PE stream per unit i: [scores(i+SKEW), pv(i)] so the PE always
            # has slot-ready scores work while pv(i) waits on exp(i).
            for sc in range(SC):
                for half in (0, 1):
                    v_piece(sc, half)
            qkts = {0: qk_proj(0)}
            units = [(oc, hh, kc) for oc in range(OC) for hh in range(2)
                     for kc in range(SC)]
            NU = len(units)
            SKEW = 2
            st_tiles = {}
            pvq_map = {}

            def emit_scores(i):
                oc, hh, kc = units[i]
                p0 = hh * 64
                qkt = qkts[oc]
                stt = st_ps.tile([128, S], F32, tag="st", name=f"st{i}")
                for qt in range(QT):
                    nc.tensor.matmul(
                        stt[:, qt * NT:(qt + 1) * NT],
                        qkt["k"][p0:p0 + 64, kc * 128:(kc + 1) * 128],
                        qkt["q"][p0:p0 + 64, qt * NT:(qt + 1) * NT],
                        tile_position=(p0, 0),
                    )
                st_tiles[i] = stt

            def emit_epilogue(oc, hh):
                gh = 2 * oc + hh
                pvq = pvq_map.pop((oc, hh))
                pvs = epi_pool.tile([HW, S], F32, tag="pvs", name="pvs", bufs=3)
                for qt in range(QT):
                    nc.vector.tensor_copy(
                        pvs[:, qt * NT:(qt + 1) * NT], pvq[qt][:]
                    )
                # Z row -> [128, 8] partition-scatter (p-major), reciprocal,
                # bounce through DRAM for the partition-broadcast read.
                zp = epi_pool.tile([128, SC], F32, tag="zp", name="zp", bufs=4)
                nc.gpsimd.dma_start(
                    zp[:], pvs[HD:HW, :].rearrange("o (p c) -> o p c", c=SC)
                )
                nc.vector.reciprocal(zp[:], zp[:])
                rzd = dram_pool.tile([S], F32, tag="rzd", name="rzd", bufs=4)
                nc.gpsimd.dma_start(rzd.rearrange("(p c) -> p c", c=SC), zp[:])
                zb = epi_pool.tile([HD, S], F32, tag="zb", name="zb", bufs=3)
                nc.gpsimd.dma_start(zb[:], rzd[:].partition_broadcast(HD))
                oh = epi_pool.tile([HD, S], F32, tag="oh", name="oh", bufs=3)
                nc.vector.tensor_mul(oh[:], pvs[0:HD, :], zb[:])
                nc.sync.dma_start(outT[gh * HD:(gh + 1) * HD, :], oh[:])

            for i in range(SKEW):
                emit_scores(i)
            for i, (oc, hh, kc) in enumerate(units):
                if i + SKEW < NU:
                    emit_scores(i + SKEW)
                stt = st_tiles.pop(i)
                ett = et_pool.tile([128, S], F32R, tag="et", name=f"et{i}")
                nc.scalar.activation(
                    ett[:],
                    stt[:],
                    mybir.ActivationFunctionType.Exp,
                    bias=mb_t[:, kc:kc + 1],
                    scale=1.0 / np.sqrt(HD),
                )
                gh = 2 * oc + hh
                if kc == 0:
                    pvq_map[(oc, hh)] = [
                        pv_ps.tile([HW, NT], F32, tag="pv", name=f"pv{gh}_{qt}")
                        for qt in range(QT)
                    ]
                pvq = pvq_map[(oc, hh)]
                for qt in range(QT):
                    nc.tensor.matmul(
                        pvq[qt][:],
                        vaug[kc][:, gh * HW:(gh + 1) * HW],
                        ett[:, qt * NT:(qt + 1) * NT],
                        start=(kc == 0),
                        stop=(kc == SC - 1),
                    )
                if kc == SC - 1:
                    emit_epilogue(oc, hh)
                piece = {(0, 6): 0, (1, 0): 1, (1, 2): 2, (1, 4): 3}.get((hh, kc))
                if piece is not None and oc + 1 < OC:
                    if piece == 0:
                        qkts[oc + 1] = qk_alloc(oc + 1)
                        qkts.pop(oc - 1, None)
                    pname, pqt = [("q", 0), ("q", 1), ("k", 0), ("k", 1)][piece]
                    qk_piece(oc + 1, qkts[oc + 1], pname, pqt)

    nc.compile()
    return nc


_NC = None


def _get_nc():
    global _NC
    if _NC is None:
        _NC = build()
    return _NC


def _in_maps(x, mask, Wq, bq, Wk, bk, Wv, bv):
    x = np.asarray(x, dtype=np.float32)
    mask = np.asarray(mask)
    wqT = np.ascontiguousarray(np.asarray(Wq, dtype=np.float32).T)
    wkT = np.ascontiguousarray(np.asarray(Wk, dtype=np.float32).T)
    wvT = np.ascontiguousarray(np.asarray(Wv, dtype=np.float32).T)
    bq = np.asarray(bq, dtype=np.float32)
    bk = np.asarray(bk, dtype=np.float32)
    bvb = np.ascontiguousarray(
        np.broadcast_to(np.asarray(bv, dtype=np.float32), (128, D))
    )
    maps = []
    for c in range(N_CORES):
        maps.append(
            {
                "xT": np.ascontiguousarray(x[c].T),
                "wqT": wqT,
                "wkT": wkT,
                "wvT": wvT,
                "bq": bq,
                "bk": bk,
                "bvb": bvb,
                "mb": (-10000.0 * (1.0 - mask[c].astype(np.float32))).astype(
                    np.float32
                ),
            }
        )
    return maps


def run(inputs, trace=False, **kw):
    nc = _get_nc()
    res = run_bass_kernel_spmd(
        nc, _in_maps(**inputs), list(range(N_CORES)), trace=trace, **kw
    )
    out = np.stack(
        [np.ascontiguousarray(res.results[c]["outT"].T) for c in range(N_CORES)]
    ).astype(np.float32)
    return out, res


def kernel(**inputs):
    out, _ = run(inputs)
    return out



# revision 7
# speedup vs baseline: 1.3107x; 1.3107x over previous
"""Multi-headed self-attention (B=8, S=1024, D=768, H=12) on 8 TRN2 cores.

Sharding: data-parallel over batch -- core i computes batch element i.
Per-core kernel, bf16 matmul operands (fp32 PSUM accumulate):
    Qt = (Wq @ x.T + bq)      [D, S] bf16  (head dim on partitions)
    Kt = (Wk @ x.T + bk)      [D, S] bf16
    Vaug[sc] = (x @ Wv.T + bv) per key chunk, head-interleaved with a
               ones column per head: [128, H*65] bf16
    St_h[kc] = Kt_h^T @ Qt_h       -> scores [k=128, q=1024] (PSUM f32)
    Et = exp(St/8 + maskbias[k])   (ACT, bf16 out)
    PV_h[qc] += Et[kc][:, qc]^T-as-weights @ Vaug_h[kc]  -> [q=128, 65]
               (q on partitions; col 64 accumulates Z = sum_k Et)
    out_h[qc] = PV[:, 0:64] * (1/Z)[q]   (per-partition scalar mult)
Output written directly in [S, D] layout -- no transposes anywhere.
"""

import numpy as np

import concourse.bacc as bacc
import concourse.tile as tile
from concourse import mybir
from concourse.bass_utils import run_bass_kernel_spmd

B, S, D, H = 8, 1024, 768, 12
HD = D // H  # 64
N_CORES = 8
SC = S // 128  # 8 key chunks
OC = D // 128  # 6 head-pair blocks
DC = D // 128  # 6 contraction chunks
NT = 512  # PSUM-bank-limited moving tile (512 fp32 out)
QT = S // NT  # 2
QC = S // 128  # 8 query chunks for PV
F32 = mybir.dt.float32
BF16 = mybir.dt.bfloat16

HW = HD + 1  # per-head V width incl. ones column
PK = D + OC + OC + SC  # packed const cols: bvb | bq | bk | mb


def build():
    nc = bacc.Bacc("TRN2", target_bir_lowering=False, debug=False, num_devices=N_CORES)
    xT = nc.dram_tensor("xT", [D, S], BF16, kind="ExternalInput").ap()
    wqT = nc.dram_tensor("wqT", [D, D], BF16, kind="ExternalInput").ap()
    wkT = nc.dram_tensor("wkT", [D, D], BF16, kind="ExternalInput").ap()
    wvT = nc.dram_tensor("wvT", [D, D], BF16, kind="ExternalInput").ap()
    pk = nc.dram_tensor("pk", [128, PK], F32, kind="ExternalInput").ap()
    outD = nc.dram_tensor("outD", [S, D], F32, kind="ExternalOutput").ap()

    with tile.TileContext(nc) as tc:
        with (
            tc.tile_pool(name="const", bufs=1) as const,
            tc.tile_pool(name="qk", bufs=2) as qk_pool,
            tc.tile_pool(name="et", bufs=6) as et_pool,
            tc.tile_pool(name="epi", bufs=3) as epi_pool,
            tc.tile_pool(name="st", bufs=2, space="PSUM") as st_ps,
            tc.tile_pool(name="tmp", bufs=2, space="PSUM") as tmp_ps,
            tc.tile_pool(name="pv", bufs=1, space="PSUM") as pv_ps,
        ):
            # ---------- input loads, spread across issue queues ----------
            xt = [const.tile([128, S], BF16, tag=f"xt{c}", name=f"xt{c}") for c in range(DC)]
            wq = [const.tile([128, D], BF16, tag=f"wq{c}", name=f"wq{c}") for c in range(DC)]
            wk = [const.tile([128, D], BF16, tag=f"wk{c}", name=f"wk{c}") for c in range(DC)]
            wv = [const.tile([128, D], BF16, tag=f"wv{c}", name=f"wv{c}") for c in range(DC)]
            pk_t = const.tile([128, PK], F32, tag="pk")
            bvb_t = pk_t[:, 0:D]
            bq_t = pk_t[:, D:D + OC]
            bk_t = pk_t[:, D + OC:D + 2 * OC]
            mb_t = pk_t[:, D + 2 * OC:PK]
            nc.sync.dma_start(pk_t[:], pk[:])
            for c in range(DC):
                nc.sync.dma_start(xt[c][:], xT[c * 128:(c + 1) * 128, :])
            for c in range(DC):
                nc.gpsimd.dma_start(wq[c][:], wqT[c * 128:(c + 1) * 128, :])
            for c in range(DC):
                nc.scalar.dma_start(wk[c][:], wkT[c * 128:(c + 1) * 128, :])
            for c in range(DC):
                nc.gpsimd.dma_start(wv[c][:], wvT[c * 128:(c + 1) * 128, :])
            # tiny dummy exp pulls the ~2.7us ACT table load off the
            # critical path
            warm = const.tile([128, 1], F32, tag="warm")
            nc.scalar.activation(
                warm[:], mb_t[:, 0:1], mybir.ActivationFunctionType.Exp
            )

            # ---------- PE warm-up (HAM ramp) during the DMA phase ----------
            wt = const.tile([128, NT], BF16, tag="wt")
            nc.vector.memset(wt[:], 0.0)
            for w in range(12):
                dm = tmp_ps.tile([128, NT], F32, tag="tmp", name=f"dm{w}")
                nc.tensor.matmul(
                    dm[:], wt[:, 0:128], wt[:], start=True, stop=True,
                    skip_group_check=True,
                )

            # ---------- V projection -> vaug [sc][128, H*65] bf16 ----------
            vaug = [const.tile([128, H * HW], BF16, tag=f"va{sc}", name=f"va{sc}") for sc in range(SC)]
            for sc in range(SC):
                ones_cols = vaug[sc][:].rearrange("p (h w) -> p h w", h=H)[:, :, HD:HW]
                nc.vector.memset(ones_cols, 1.0)

            def v_piece(sc, ocb):
                # heads (2*ocb, 2*ocb+1): W cols ocb*128:(ocb+1)*128
                n0 = ocb * 128
                vp = tmp_ps.tile([128, NT], F32, tag="tmp", name=f"vp{sc}_{ocb}")
                for c in range(DC):
                    nc.tensor.matmul(
                        vp[:, 0:128],
                        xt[c][:, sc * 128:(sc + 1) * 128],
                        wv[c][:, n0:n0 + 128],
                        start=(c == 0),
                        stop=(c == DC - 1),
                    )
                nc.vector.tensor_add(
                    vaug[sc][:].rearrange("p (h w) -> p h w", h=H)[:, 2 * ocb:2 * ocb + 2, 0:HD],
                    vp[:, 0:128].rearrange("p (h w) -> p h w", w=HD),
                    bvb_t[:, n0:n0 + 128].rearrange("p (h w) -> p h w", w=HD),
                )

            # ---------- Q/K projection pieces ----------
            wmap = {"q": (wq, bq_t), "k": (wk, bk_t)}

            def qk_alloc(oc):
                return {
                    name: qk_pool.tile([128, S], BF16, tag=name, name=f"{name}t{oc}")
                    for name in ("q", "k")
                }

            def qk_piece(oc, dsts, name, qt):
                w_t, b_t = wmap[name]
                p = tmp_ps.tile([128, NT], F32, tag="tmp", name=f"qkp{name}{qt}")
                for c in range(DC):
                    nc.tensor.matmul(
                        p[:],
                        w_t[c][:, oc * 128:(oc + 1) * 128],
                        xt[c][:, qt * NT:(qt + 1) * NT],
                        start=(c == 0),
                        stop=(c == DC - 1),
                    )
                nc.vector.tensor_scalar_add(
                    dsts[name][:, qt * NT:(qt + 1) * NT], p[:], b_t[:, oc:oc + 1]
                )

            def qk_proj(oc):
                dsts = qk_alloc(oc)
                for name in ("q", "k"):
                    for qt in range(QT):
                        qk_piece(oc, dsts, name, qt)
                return dsts

            # ---------- attention units: (oc, hh, kc), kc inner ----------
            qkts = {0: qk_proj(0)}
            units = [(oc, hh, kc) for oc in range(OC) for hh in range(2)
                     for kc in range(SC)]
            NU = len(units)
            SKEW = 1
            st_tiles = {}
            pv_map = {}

            def emit_scores(i):
                oc, hh, kc = units[i]
                p0 = hh * 64
                qkt = qkts[oc]
                stt = st_ps.tile([128, S], F32, tag="st", name=f"st{i}")
                for qt in range(QT):
                    nc.tensor.matmul(
                        stt[:, qt * NT:(qt + 1) * NT],
                        qkt["k"][p0:p0 + 64, kc * 128:(kc + 1) * 128],
                        qkt["q"][p0:p0 + 64, qt * NT:(qt + 1) * NT],
                        tile_position=(p0, 0),
                    )
                st_tiles[i] = stt

            def emit_epilogue(oc, hh):
                gh = 2 * oc + hh
                t1, t2 = pv_map.pop((oc, hh))
                # drain PSUM fast with two wide copies so the single pv
                # buffer frees before the next head's first matmul (gpsimd
                # cannot read PSUM, and per-qc mults would serialize)
                pvs = epi_pool.tile([128, QC * HW], F32, tag="pvs", name="pvs", bufs=3)
                nc.vector.tensor_copy(pvs[:, 0:(QC - 1) * HW], t1[:])
                nc.vector.tensor_copy(pvs[:, (QC - 1) * HW:QC * HW], t2[:])
                # 1/Z per query (q on partitions -> per-partition scalar)
                zr = epi_pool.tile([128, QC], F32, tag="zr", name="zr", bufs=4)
                nc.vector.reciprocal(
                    zr[:], pvs[:].rearrange("p (c w) -> p c w", w=HW)[:, :, HD]
                )
                oh = epi_pool.tile([128, QC * HD], F32, tag="oh", name="oh", bufs=3)
                for qc in range(QC):
                    eng = nc.vector if qc % 2 == 0 else nc.gpsimd
                    eng.tensor_scalar_mul(
                        oh[:, qc * HD:(qc + 1) * HD],
                        pvs[:, qc * HW:qc * HW + HD],
                        zr[:, qc:qc + 1],
                    )
                dst = outD.rearrange("(c p) (g w) -> p c g w", p=128, w=HD)[:, :, gh, :]
                ohr = oh[:].rearrange("p (c w) -> p c w", w=HD)
                nc.sync.dma_start(dst[:, 0:QC // 2], ohr[:, 0:QC // 2])
                nc.gpsimd.dma_start(dst[:, QC // 2:QC], ohr[:, QC // 2:QC])

            # V pieces injected just-in-time: piece (sc, ocb) is read first
            # by unit (ocb, 0, sc); emit inside the previous half-oc block.
            v_sched = {}
            for ocb in range(OC):
                for sc in range(SC):
                    if ocb == 0:
                        u = max(sc - 2, 0)
                    else:
                        u = ocb * 16 - 8 + sc
                    v_sched.setdefault(u, []).append((sc, ocb))

            v_piece(0, 0)
            v_piece(1, 0)
            for i in range(SKEW + 1):
                emit_scores(i)
            for i, (oc, hh, kc) in enumerate(units):
                if i + SKEW + 1 < NU:
                    emit_scores(i + SKEW + 1)
                stt = st_tiles.pop(i)
                ett = et_pool.tile([128, S], BF16, tag="et", name=f"et{i}")
                nc.scalar.activation(
                    ett[:],
                    stt[:],
                    mybir.ActivationFunctionType.Exp,
                    bias=mb_t[:, kc:kc + 1],
                    scale=1.0 / np.sqrt(HD),
                )
                gh = 2 * oc + hh
                if kc == 0:
                    t1 = pv_ps.tile([128, (QC - 1) * HW], F32, tag="pvt1", name=f"pvt1_{gh}")
                    t2 = pv_ps.tile([128, HW], F32, tag="pvt2", name=f"pvt2_{gh}")
                    pv_map[(oc, hh)] = (t1, t2)
                t1, t2 = pv_map[(oc, hh)]
                # PSUM start=True resets the whole bank's has_written bits, so
                # exactly one start (and one stop) per bank: qc0 for t1's
                # bank, qc7 for t2's. Later first-writes land on cleared bits
                # and overwrite; subsequent kc iterations accumulate.
                for qc in range(QC):
                    out_ap = (
                        t1[:, qc * HW:(qc + 1) * HW] if qc < QC - 1 else t2[:]
                    )
                    nc.tensor.matmul(
                        out_ap,
                        ett[:, qc * 128:(qc + 1) * 128],
                        vaug[kc][:, gh * HW:(gh + 1) * HW],
                        start=(kc == 0 and qc in (0, QC - 1)),
                        stop=(kc == SC - 1 and qc in (QC - 2, QC - 1)),
                        skip_group_check=True,
                    )
                if kc == SC - 1:
                    emit_epilogue(oc, hh)
                for sc, ocb in v_sched.get(i, ()):
                    v_piece(sc, ocb)
                piece = {(0, 6): 0, (1, 0): 1, (1, 2): 2, (1, 4): 3}.get((hh, kc))
                if piece is not None and oc + 1 < OC:
                    if piece == 0:
                        qkts[oc + 1] = qk_alloc(oc + 1)
                        qkts.pop(oc - 1, None)
                    pname, pqt = [("q", 0), ("q", 1), ("k", 0), ("k", 1)][piece]
                    qk_piece(oc + 1, qkts[oc + 1], pname, pqt)

    nc.compile()
    return nc


_NC = None


def _get_nc():
    global _NC
    if _NC is None:
        _NC = build()
    return _NC


def _bf16(a):
    import ml_dtypes

    return np.asarray(a, dtype=np.float32).astype(ml_dtypes.bfloat16)


def _in_maps(x, mask, Wq, bq, Wk, bk, Wv, bv):
    x = np.asarray(x, dtype=np.float32)
    mask = np.asarray(mask)
    wqT = _bf16(np.asarray(Wq, dtype=np.float32).T)
    wkT = _bf16(np.asarray(Wk, dtype=np.float32).T)
    wvT = _bf16(np.asarray(Wv, dtype=np.float32).T)
    maps = []
    for c in range(N_CORES):
        pk = np.zeros((128, PK), dtype=np.float32)
        pk[:, 0:D] = np.asarray(bv, dtype=np.float32)[None, :]
        pk[:, D:D + OC] = np.asarray(bq, dtype=np.float32).reshape(OC, 128).T
        pk[:, D + OC:D + 2 * OC] = np.asarray(bk, dtype=np.float32).reshape(OC, 128).T
        mbc = -10000.0 * (1.0 - mask[c].astype(np.float32))
        pk[:, D + 2 * OC:PK] = mbc.reshape(SC, 128).T
        maps.append(
            {
                "xT": _bf16(x[c].T),
                "wqT": wqT,
                "wkT": wkT,
                "wvT": wvT,
                "pk": pk,
            }
        )
    return maps


def run(inputs, trace=False, **kw):
    nc = _get_nc()
    res = run_bass_kernel_spmd(
        nc, _in_maps(**inputs), list(range(N_CORES)), trace=trace, **kw
    )
    out = np.stack(
        [np.asarray(res.results[c]["outD"]) for c in range(N_CORES)]
    ).astype(np.float32)
    return out, res


def kernel(**inputs):
    out, _ = run(inputs)
    return out


# revision 12
# speedup vs baseline: 1.3617x; 1.0389x over previous
"""Multi-headed self-attention (B=8, S=1024, D=768, H=12) on 8 TRN2 cores.

Sharding: data-parallel over batch -- core i computes batch element i.
Per-core kernel, bf16 matmul operands (fp32 PSUM accumulate):
    Qt = (Wq @ x.T + bq)      [D, S] bf16  (head dim on partitions)
    Kt = (Wk @ x.T + bk)      [D, S] bf16
    Vaug[sc] = (x @ Wv.T + bv) per key chunk, head-interleaved with a
               ones column per head: [128, H*65] bf16
    St_h[kc] = Kt_h^T @ Qt_h       -> scores [k=128, q=1024] (PSUM f32)
    Et = exp(St/8 + maskbias[k])   (ACT, bf16 out)
    PV_h[qc] += Et[kc][:, qc]^T-as-weights @ Vaug_h[kc]  -> [q=128, 65]
               (q on partitions; col 64 accumulates Z = sum_k Et)
    out_h[qc] = PV[:, 0:64] * (1/Z)[q]   (per-partition scalar mult)
Output written directly in [S, D] layout -- no transposes anywhere.
"""

import numpy as np

import concourse.bacc as bacc
import concourse.bass as bass
import concourse.tile as tile
from concourse import mybir
from concourse.bass_utils import run_bass_kernel_spmd

B, S, D, H = 8, 1024, 768, 12
HD = D // H  # 64
N_CORES = 8
SC = S // 128  # 8 key chunks
OC = D // 128  # 6 head-pair blocks
DC = D // 128  # 6 contraction chunks
NT = 512  # PSUM-bank-limited moving tile (512 fp32 out)
QT = S // NT  # 2
QC = S // 128  # 8 query chunks for PV
F32 = mybir.dt.float32
BF16 = mybir.dt.bfloat16

HW = HD + 1  # per-head V width incl. ones column
PK = D + OC + OC + SC  # packed const cols: bvb | bq | bk | mb


def build():
    nc = bacc.Bacc("TRN2", target_bir_lowering=False, debug=False, num_devices=N_CORES)
    xT = nc.dram_tensor("xT", [D, S], BF16, kind="ExternalInput").ap()
    wqT = nc.dram_tensor("wqT", [D, D], BF16, kind="ExternalInput").ap()
    wkT = nc.dram_tensor("wkT", [D, D], BF16, kind="ExternalInput").ap()
    wvT = nc.dram_tensor("wvT", [D, D], BF16, kind="ExternalInput").ap()
    pk = nc.dram_tensor("pk", [128, PK], F32, kind="ExternalInput").ap()
    outD = nc.dram_tensor("outD", [S, D], F32, kind="ExternalOutput").ap()

    with tile.TileContext(nc) as tc:
        with (
            tc.tile_pool(name="const", bufs=1) as const,
            tc.tile_pool(name="qk", bufs=2) as qk_pool,
            tc.tile_pool(name="et", bufs=6) as et_pool,
            tc.tile_pool(name="epi", bufs=3) as epi_pool,
            tc.tile_pool(name="st", bufs=2, space="PSUM") as st_ps,
            tc.tile_pool(name="tmp", bufs=2, space="PSUM") as tmp_ps,
            tc.tile_pool(name="pv", bufs=1, space="PSUM") as pv_ps,
        ):
            # ---------- input loads, spread across issue queues ----------
            xt = [const.tile([128, S], BF16, tag=f"xt{c}", name=f"xt{c}") for c in range(DC)]
            wq = [const.tile([128, D], BF16, tag=f"wq{c}", name=f"wq{c}") for c in range(DC)]
            wk = [const.tile([128, D], BF16, tag=f"wk{c}", name=f"wk{c}") for c in range(DC)]
            wv = [const.tile([128, D], BF16, tag=f"wv{c}", name=f"wv{c}") for c in range(DC)]
            pk_t = const.tile([128, PK], F32, tag="pk")
            bvb_t = pk_t[:, 0:D]
            bq_t = pk_t[:, D:D + OC]
            bk_t = pk_t[:, D + OC:D + 2 * OC]
            mb_t = pk_t[:, D + 2 * OC:PK]
            for c in range(DC):
                nc.sync.dma_start(xt[c][:], xT[c * 128:(c + 1) * 128, :])
            nc.sync.dma_start(pk_t[:], pk[:])
            for c in range(DC):
                nc.gpsimd.dma_start(wq[c][:], wqT[c * 128:(c + 1) * 128, :])
            for c in range(DC):
                nc.scalar.dma_start(wk[c][:], wkT[c * 128:(c + 1) * 128, :])
            for c in range(DC):
                nc.gpsimd.dma_start(wv[c][:], wvT[c * 128:(c + 1) * 128, :])
            # tiny dummy exp pulls the ~2.7us ACT table load off the
            # critical path
            warm = const.tile([128, 1], F32, tag="warm")
            nc.scalar.activation(
                warm[:], mb_t[:, 0:1], mybir.ActivationFunctionType.Exp
            )

            # ---------- PE warm-up (HAM ramp) during the DMA phase.
            # Must bridge seamlessly into the DMA-chasing qk matmuls: an
            # idle PE re-throttles to half rate with long hysteresis.
            wt = const.tile([128, NT], BF16, tag="wt")
            nc.vector.memset(wt[:], 0.0)
            for w in range(18):
                dm = tmp_ps.tile([128, NT], F32, tag="tmp", name=f"dm{w}")
                nc.tensor.matmul(
                    dm[:], wt[:, 0:128], wt[:], start=True, stop=True,
                    skip_group_check=True,
                )

            # ---------- V projection -> vaug [sc][128, H*65] bf16 ----------
            vaug = [const.tile([128, H * HW], BF16, tag=f"va{sc}", name=f"va{sc}") for sc in range(SC)]
            for sc in range(SC):
                ones_cols = vaug[sc][:].rearrange("p (h w) -> p h w", h=H)[:, :, HD:HW]
                nc.vector.memset(ones_cols, 1.0)

            def v_piece(sc, half):
                # big-N matmuls: small-N MMs are latency-bound (no ldw-opt)
                n0, n1, h0, h1 = ((0, 512, 0, 8), (512, 768, 8, 12))[half]
                vp = tmp_ps.tile([128, NT], F32, tag="tmp", name=f"vp{sc}_{half}")
                for c in range(DC):
                    nc.tensor.matmul(
                        vp[:, : n1 - n0],
                        xt[c][:, sc * 128:(sc + 1) * 128],
                        wv[c][:, n0:n1],
                        start=(c == 0),
                        stop=(c == DC - 1),
                    )
                nc.vector.tensor_add(
                    vaug[sc][:].rearrange("p (h w) -> p h w", h=H)[:, h0:h1, 0:HD],
                    vp[:, : n1 - n0].rearrange("p (h w) -> p h w", w=HD),
                    bvb_t[:, n0:n1].rearrange("p (h w) -> p h w", w=HD),
                )

            # ---------- Q/K projection pieces ----------
            wmap = {"q": (wq, bq_t), "k": (wk, bk_t)}

            def qk_alloc(oc):
                return {
                    name: qk_pool.tile([128, S], BF16, tag=name, name=f"{name}t{oc}")
                    for name in ("q", "k")
                }

            def qk_piece(oc, dsts, name, qt):
                w_t, b_t = wmap[name]
                p = tmp_ps.tile([128, NT], F32, tag="tmp", name=f"qkp{name}{qt}")
                for c in range(DC):
                    nc.tensor.matmul(
                        p[:],
                        w_t[c][:, oc * 128:(oc + 1) * 128],
                        xt[c][:, qt * NT:(qt + 1) * NT],
                        start=(c == 0),
                        stop=(c == DC - 1),
                    )
                nc.vector.tensor_scalar_add(
                    dsts[name][:, qt * NT:(qt + 1) * NT], p[:], b_t[:, oc:oc + 1]
                )

            def qk_proj(oc):
                dsts = qk_alloc(oc)
                for name in ("q", "k"):
                    for qt in range(QT):
                        qk_piece(oc, dsts, name, qt)
                return dsts

            # ---------- attention units: (oc, hh, kc), kc inner ----------
            qkts = {0: qk_proj(0)}
            units = [(oc, hh, kc) for oc in range(OC) for hh in range(2)
                     for kc in range(SC)]
            NU = len(units)
            SKEW = 1
            st_tiles = {}
            pv_map = {}

            def emit_scores(i):
                oc, hh, kc = units[i]
                p0 = hh * 64
                qkt = qkts[oc]
                stt = st_ps.tile([128, S], F32, tag="st", name=f"st{i}")
                for qt in range(QT):
                    nc.tensor.matmul(
                        stt[:, qt * NT:(qt + 1) * NT],
                        qkt["k"][p0:p0 + 64, kc * 128:(kc + 1) * 128],
                        qkt["q"][p0:p0 + 64, qt * NT:(qt + 1) * NT],
                        tile_position=(p0, 0),
                    )
                st_tiles[i] = stt

            def emit_epilogue(oc, hh):
                gh = 2 * oc + hh
                t1, t2 = pv_map.pop((oc, hh))
                # drain PSUM fast with two wide copies so the single pv
                # buffer frees before the next head's first matmul (gpsimd
                # cannot read PSUM, and per-qc mults would serialize)
                pvs = epi_pool.tile([128, QC * HW], F32, tag="pvs", name="pvs", bufs=3)
                nc.vector.tensor_copy(pvs[:, 0:(QC - 1) * HW], t1[:])
                nc.vector.tensor_copy(pvs[:, (QC - 1) * HW:QC * HW], t2[:])
                # 1/Z per query (q on partitions -> per-partition scalar)
                zr = epi_pool.tile([128, QC], F32, tag="zr", name="zr", bufs=4)
                nc.vector.reciprocal(
                    zr[:], pvs[:].rearrange("p (c w) -> p c w", w=HW)[:, :, HD]
                )
                oh = epi_pool.tile([128, QC * HD], F32, tag="oh", name="oh", bufs=3)
                # single fused multiply: broadcast 1/Z along the head dim
                pv_v = pvs[:].rearrange("p (c w) -> p c w", w=HW)[:, :, 0:HD]
                zr_v = zr[:].rearrange("p (c o) -> p c o", o=1)
                pv_b, zr_b = bass.broadcast_tensor_aps(pv_v, zr_v)
                nc.vector.tensor_mul(
                    oh[:].rearrange("p (c w) -> p c w", w=HD), pv_b, zr_b
                )
                dst = outD.rearrange("(c p) (g w) -> p c g w", p=128, w=HD)[:, :, gh, :]
                ohr = oh[:].rearrange("p (c w) -> p c w", w=HD)
                nc.sync.dma_start(dst[:, 0:QC // 2], ohr[:, 0:QC // 2])
                nc.gpsimd.dma_start(dst[:, QC // 2:QC], ohr[:, QC // 2:QC])

            # V pieces injected just-in-time: piece (sc, half0) is read first
            # by unit (0, 0, sc); half1 (heads 8-11) not read until unit 64.
            v_sched = {}
            for sc in range(2, SC):
                v_sched.setdefault(sc - 2, []).append((sc, 0))
            for sc in range(SC):
                v_sched.setdefault(52 + sc, []).append((sc, 1))

            v_piece(0, 0)
            v_piece(1, 0)
            for i in range(SKEW + 1):
                emit_scores(i)
            for i, (oc, hh, kc) in enumerate(units):
                if i + SKEW + 1 < NU:
                    emit_scores(i + SKEW + 1)
                stt = st_tiles.pop(i)
                ett = et_pool.tile([128, S], BF16, tag="et", name=f"et{i}")
                nc.scalar.activation(
                    ett[:],
                    stt[:],
                    mybir.ActivationFunctionType.Exp,
                    bias=mb_t[:, kc:kc + 1],
                    scale=1.0 / np.sqrt(HD),
                )
                gh = 2 * oc + hh
                if kc == 0:
                    t1 = pv_ps.tile([128, (QC - 1) * HW], F32, tag="pvt1", name=f"pvt1_{gh}")
                    t2 = pv_ps.tile([128, HW], F32, tag="pvt2", name=f"pvt2_{gh}")
                    pv_map[(oc, hh)] = (t1, t2)
                t1, t2 = pv_map[(oc, hh)]
                # PSUM start=True resets the whole bank's has_written bits, so
                # exactly one start (and one stop) per bank: qc0 for t1's
                # bank, qc7 for t2's. Later first-writes land on cleared bits
                # and overwrite; subsequent kc iterations accumulate.
                for qc in range(QC):
                    out_ap = (
                        t1[:, qc * HW:(qc + 1) * HW] if qc < QC - 1 else t2[:]
                    )
                    nc.tensor.matmul(
                        out_ap,
                        ett[:, qc * 128:(qc + 1) * 128],
                        vaug[kc][:, gh * HW:(gh + 1) * HW],
                        start=(kc == 0 and qc in (0, QC - 1)),
                        stop=(kc == SC - 1 and qc in (QC - 2, QC - 1)),
                        skip_group_check=True,
                    )
                if kc == SC - 1:
                    emit_epilogue(oc, hh)
                for sc, half in v_sched.get(i, ()):
                    v_piece(sc, half)
                piece = {(0, 6): 0, (1, 0): 1, (1, 2): 2, (1, 4): 3}.get((hh, kc))
                if piece is not None and oc + 1 < OC:
                    if piece == 0:
                        qkts[oc + 1] = qk_alloc(oc + 1)
                        qkts.pop(oc - 1, None)
                    pname, pqt = [("q", 0), ("q", 1), ("k", 0), ("k", 1)][piece]
                    qk_piece(oc + 1, qkts[oc + 1], pname, pqt)

    nc.compile()
    return nc


_NC = None


def _get_nc():
    global _NC
    if _NC is None:
        _NC = build()
    return _NC


def _bf16(a):
    import ml_dtypes

    return np.asarray(a, dtype=np.float32).astype(ml_dtypes.bfloat16)


def _in_maps(x, mask, Wq, bq, Wk, bk, Wv, bv):
    x = np.asarray(x, dtype=np.float32)
    mask = np.asarray(mask)
    wqT = _bf16(np.asarray(Wq, dtype=np.float32).T)
    wkT = _bf16(np.asarray(Wk, dtype=np.float32).T)
    wvT = _bf16(np.asarray(Wv, dtype=np.float32).T)
    maps = []
    for c in range(N_CORES):
        pk = np.zeros((128, PK), dtype=np.float32)
        pk[:, 0:D] = np.asarray(bv, dtype=np.float32)[None, :]
        pk[:, D:D + OC] = np.asarray(bq, dtype=np.float32).reshape(OC, 128).T
        pk[:, D + OC:D + 2 * OC] = np.asarray(bk, dtype=np.float32).reshape(OC, 128).T
        mbc = -10000.0 * (1.0 - mask[c].astype(np.float32))
        pk[:, D + 2 * OC:PK] = mbc.reshape(SC, 128).T
        maps.append(
            {
                "xT": _bf16(x[c].T),
                "wqT": wqT,
                "wkT": wkT,
                "wvT": wvT,
                "pk": pk,
            }
        )
    return maps


def run(inputs, trace=False, **kw):
    nc = _get_nc()
    res = run_bass_kernel_spmd(
        nc, _in_maps(**inputs), list(range(N_CORES)), trace=trace, **kw
    )
    out = np.stack(
        [np.asarray(res.results[c]["outD"]) for c in range(N_CORES)]
    ).astype(np.float32)
    return out, res


def kernel(**inputs):
    out, _ = run(inputs)
    return out
